# revision 1
# baseline (speedup 1.0000x reference)
"""Trainium2 kernel for nn_MeanSquaredError2: MSE between argmax-decoded
heatmap coordinates and targets.

loss = sum_{b,j} [(px - tpx)^2 + (py - tpy)^2] / (B*NJ)
  where idx = argmax(h[b,j]), px = (idx%14)/16, py = (idx//14)/16 and
  (tpx, tpy) follow the reference's concat-then-reshape pairing of t.
Inputs o and v do not affect the result (USE_VISIBILITY=False).

Pure data parallel over 8 cores (2048 batches each). Per core, h streams in
16 tiles of [128 part x (14 rows x 196 pix)] at the ~360 GB/s HBM roofline;
the argmax is computed by a pack-and-max scheme using only ops this walrus
supports per engine:
  op1 (ACT):      u = h*2^20 + 1.5*2^31   (fma; quantizes h to 2^-12 steps,
                                           monotone: ulp at 1.5*2^31 is 256)
  op2 (ACT/Pool): q = u - 1.5*2^31        (exact via Sterbenz: Q*256)
  op3 (Pool tt_add, or fused into a single DVE stt (u-MAGIC)+w8):
      K = q + w8, exact: w8 in [1,223] < 256,
      w8[y*14+x] = (13-y)*16 + (13-x) + 1 (ties prefer smaller (y,x), i.e.
                                           first occurrence like jnp.argmax)
  reduce (DVE):   Kmax[row] = max_i K[row, i]
Tail (chunked, interleaved with the stream): w8 = int32(Kmax) & 255;
x = 14-(w8&15); y = 13-(w8>>4); px=x/16, py=y/16; squared errors vs targets
accumulate per partition on ACT. Host sums the 8x[128,4] partials / N.

DVE handles op3 for the EARLY tiles (fused, filling its pre-reduce idle);
Pool takes all late tiles so the closing chain never queues behind DVE's
reduce backlog. Requires bacc.Bacc (generate_event_semaphores legalizes
TRN2's one-sync-wait-per-instruction constraint).
"""
import numpy as np

B = 16384
NJ = 14
NPIX = 196
N_CORES = 8
ROWS_PER_TILE = 1792          # 128 partitions x 14 rows
K_PER_PART = 14
N_TILES = 16                  # (B/N_CORES)*NJ / ROWS_PER_TILE
ACT_OP2_TILES = 10
DVE_OP3_TILES = 6

SCALE = float(2 ** 20)
MAGIC = 1.5 * 2 ** 31

_STATE = {}


def _build(act_op2_tiles: int, dve_op3_tiles: int):
    import concourse.bacc as bacc
    import concourse.mybir as mybir
    from concourse.tile import TileContext

    F32 = mybir.dt.float32
    I32 = mybir.dt.int32
    A = mybir.AluOpType
    AF = mybir.ActivationFunctionType

    n_tiles = N_TILES
    rows = n_tiles * ROWS_PER_TILE
    ncols = n_tiles * K_PER_PART

    nc = bacc.Bacc()
    h = nc.declare_dram_parameter("h", [rows, NPIX], F32, isOutput=False)
    w = nc.declare_dram_parameter("w", [128, NPIX], F32, isOutput=False)
    tx = nc.declare_dram_parameter("tx", [128, ncols], F32, isOutput=False)
    ty = nc.declare_dram_parameter("ty", [128, ncols], F32, isOutput=False)
    mg = nc.declare_dram_parameter("mg", [128, 2], F32, isOutput=False)
    out = nc.declare_dram_parameter("part", [128, 4], F32, isOutput=True)

    with TileContext(nc) as tc:
        with tc.tile_pool(name="hpool", bufs=5) as hpool, \
             tc.tile_pool(name="upool", bufs=5) as upool, \
             tc.tile_pool(name="consts", bufs=1) as cpool, \
             tc.tile_pool(name="acc", bufs=1) as accpool:
            wt = cpool.tile([128, NPIX], F32, tag="wt")
            nc.sync.dma_start(wt[:], w[:])
            mgt = cpool.tile([128, 2], F32, tag="mgt")
            nc.sync.dma_start(mgt[:], mg[:])
            txt = cpool.tile([128, ncols], F32, tag="txt")
            nc.sync.dma_start(txt[:], tx[:])
            tyt = cpool.tile([128, ncols], F32, tag="tyt")
            nc.sync.dma_start(tyt[:], ty[:])
            kmax = accpool.tile([128, ncols], F32, tag="kmax")

            # op3: DVE takes the EARLY tiles and fuses op2 into its stt
            # ((u - MAGIC) + w8 in one pass); Pool takes all late tiles so the
            # closing chain never queues behind DVE's reduce backlog. op2 for
            # the non-fused tiles: Pool takes the first block, ACT the rest.
            pool3 = set(range(dve_op3_tiles, n_tiles))
            n_pool2 = max(n_tiles - dve_op3_tiles - act_op2_tiles, 0)
            pool2 = set(range(dve_op3_tiles, dve_op3_tiles + n_pool2))
            n_chunks = 4
            cw = ncols // n_chunks
            tiles_per_chunk = n_tiles // n_chunks
            part_sb = accpool.tile([128, n_chunks], F32, tag="part")

            def emit_tail_chunk(c):
                lo, hi = c * cw, (c + 1) * cw
                i32 = accpool.tile([128, cw], I32, tag=f"i32_{c}")
                nc.vector.tensor_copy(i32[:], kmax[:, lo:hi])
                w8 = accpool.tile([128, cw], I32, tag=f"w8_{c}")
                nc.vector.tensor_scalar(w8[:], i32[:], 255, None, op0=A.bitwise_and)
                xr = accpool.tile([128, cw], I32, tag=f"xr_{c}")
                nc.vector.tensor_scalar(xr[:], w8[:], 15, None, op0=A.bitwise_and)
                yr = accpool.tile([128, cw], I32, tag=f"yr_{c}")
                nc.vector.tensor_scalar(yr[:], w8[:], 4, None, op0=A.arith_shift_right)
                xrf = accpool.tile([128, cw], F32, tag=f"xrf_{c}")
                nc.vector.tensor_copy(xrf[:], xr[:])
                yrf = accpool.tile([128, cw], F32, tag=f"yrf_{c}")
                nc.vector.tensor_copy(yrf[:], yr[:])
                # px = (14-xr)/16 in [0, 13/16]; py = (13-yr)/16; clamped
                nc.vector.tensor_scalar(xrf[:], xrf[:], -0.0625, 0.875, op0=A.mult, op1=A.add)
                nc.vector.tensor_scalar(xrf[:], xrf[:], 0.0, 0.8125, op0=A.max, op1=A.min)
                nc.vector.tensor_scalar(yrf[:], yrf[:], -0.0625, 0.8125, op0=A.mult, op1=A.add)
                nc.vector.tensor_scalar(yrf[:], yrf[:], 0.0, 0.8125, op0=A.max, op1=A.min)
                dxy = accpool.tile([128, 2 * cw], F32, tag=f"dxy_{c}")
                nc.vector.tensor_tensor(dxy[:, :cw], xrf[:], txt[:, lo:hi], op=A.subtract)
                nc.vector.tensor_tensor(dxy[:, cw:], yrf[:], tyt[:, lo:hi], op=A.subtract)
                sq = accpool.tile([128, 2 * cw], F32, tag=f"sq_{c}")
                nc.scalar.activation(sq[:], dxy[:], AF.Square,
                                     accum_out=part_sb[:, c:c + 1])
            # work list: (row0, nrows_k, kmax_col0, engine_path). The last
            # two tiles are split into half-tiles with the final halves on
            # the fused-DVE path, shortening the serial closing chain.
            work = []
            for t in range(n_tiles):
                path = "pool" if t in pool3 else ("pool2" if t in pool2 else "dve")
                work.append((t, 0, K_PER_PART, path, (t + 1) % tiles_per_chunk == 0))
            for t, klo, nk, path, do_tail in work:
                k0 = t * K_PER_PART + klo
                ht = hpool.tile([128, K_PER_PART * NPIX], F32, tag="ht")
                # partition p owns rows t*1792 + p*14 + (klo..klo+nk); for a
                # half tile this is a strided (per-partition) row subset
                nc.sync.dma_start(
                    ht[:],
                    h[t * ROWS_PER_TILE:(t + 1) * ROWS_PER_TILE, :]
                    .rearrange("(p k) f -> p (k f)", p=128))
                u = upool.tile([128, K_PER_PART * NPIX], F32, tag="u")
                # op1: u = h*SCALE + MAGIC (single-rounding fma on ACT)
                nc.scalar.activation(u[:, :nk * NPIX], ht[:, :nk * NPIX],
                                     AF.Identity, bias=mgt[:, 0:1], scale=SCALE)
                w3 = wt.rearrange("p (o f) -> p o f", o=1).broadcast_to(
                    [128, nk, NPIX])
                u3 = u[:, :nk * NPIX].rearrange("p (k f) -> p k f", f=NPIX)
                if path in ("pool", "pool2"):
                    # op2: q = u - MAGIC (exact), in place
                    if path == "pool2":
                        nc.gpsimd.tensor_scalar(u[:, :nk * NPIX], u[:, :nk * NPIX],
                                                MAGIC, None, op0=A.subtract)
                    else:
                        nc.scalar.activation(u[:, :nk * NPIX], u[:, :nk * NPIX],
                                             AF.Identity, bias=mgt[:, 1:2], scale=1.0)
                    # op3: K = q + w8 (exact), in place
                    nc.gpsimd.tensor_tensor(u3, u3, w3, op=A.add)
                else:
                    # fused op2+op3 on DVE: K = (u - MAGIC) + w8, one pass
                    nc.vector.scalar_tensor_tensor(
                        u3, u3, MAGIC, w3, op0=A.subtract, op1=A.add)
                # reduce: Kmax per row
                nc.vector.tensor_reduce(
                    kmax[:, k0:k0 + nk],
                    u3, axis=mybir.AxisListType.X, op=A.max)
                if do_tail:
                    emit_tail_chunk((k0 + nk) // cw - 1)

            nc.sync.dma_start(out[:], part_sb[:])
    nc.finalize()
    return nc


def _weight_pattern() -> np.ndarray:
    i = np.arange(NPIX)
    y = i // 14
    x = i % 14
    wp = (13 - y) * 16 + (13 - x) + 1
    return np.broadcast_to(wp.astype(np.float32), (128, NPIX)).copy()


def _magic() -> np.ndarray:
    return np.broadcast_to(np.array([MAGIC, -MAGIC], np.float32), (128, 2)).copy()


def _targets(t_shard: np.ndarray):
    b = t_shard.shape[0]
    t2 = t_shard.reshape(b, 28)
    tpx = np.ascontiguousarray(t2[:, :14]).reshape(-1)
    tpy = np.ascontiguousarray(t2[:, 14:]).reshape(-1)
    tx = tpx.reshape(N_TILES, 128, K_PER_PART).transpose(1, 0, 2).reshape(128, -1)
    ty = tpy.reshape(N_TILES, 128, K_PER_PART).transpose(1, 0, 2).reshape(128, -1)
    return np.ascontiguousarray(tx), np.ascontiguousarray(ty)


def kernel(o: np.ndarray, h: np.ndarray, t: np.ndarray, v: np.ndarray,
           _trace: bool = False, _tmpdir: str | None = None) -> np.ndarray:
    from concourse.bass_utils import run_bass_kernel_spmd

    key = (ACT_OP2_TILES, DVE_OP3_TILES)
    if _STATE.get("key") != key:
        _STATE["nc"] = _build(*key)
        _STATE["key"] = key
    nc = _STATE["nc"]

    h = np.ascontiguousarray(np.asarray(h, dtype=np.float32))
    t = np.ascontiguousarray(np.asarray(t, dtype=np.float32))
    bs = B // N_CORES
    wp = _weight_pattern()
    mgc = _magic()
    in_maps = []
    for c in range(N_CORES):
        h_shard = h[c * bs:(c + 1) * bs].reshape(bs * NJ, NPIX)
        txv, tyv = _targets(t[c * bs:(c + 1) * bs])
        in_maps.append({"h": h_shard, "w": wp, "tx": txv, "ty": tyv, "mg": mgc})

    res = run_bass_kernel_spmd(
        nc, in_maps, list(range(N_CORES)),
        trace=_trace, tmpdir=_tmpdir)
    _STATE["last_result"] = res
    total = np.float64(0.0)
    for c in range(N_CORES):
        total += np.asarray(res.results[c]["part"], dtype=np.float64).sum()
    n = np.float32(B * NJ)
    return np.float32(np.float32(total) / n)



# revision 3
# speedup vs baseline: 1.5107x; 1.5107x over previous
"""Trainium2 kernel for nn_MeanSquaredError2: MSE between argmax-decoded
heatmap coordinates and targets.

loss = sum_{b,j} [(px - tpx)^2 + (py - tpy)^2] / (B*NJ)
  where idx = argmax(h[b,j]), px = (idx%14)/16, py = (idx//14)/16 and
  (tpx, tpy) follow the reference's concat-then-reshape pairing of t.
Inputs o and v do not affect the result (USE_VISIBILITY=False).

Pure data parallel over 8 cores (2048 batches each). Per core, h streams in
16 tiles of [128 part x (14 rows x 196 pix)]; the whole
quantize+pack+row-argmax runs as ONE custom DVE instruction per tile
(1 elem/cycle), leaving the kernel DMA-bound:

  custom op QPACK_ROWMAX_ANT (registered into dve_ops.OPS at import):
      u = (h*2^16 + MAGIC)          # rounds h*2^16 to the 256-grid
      q = u - MAGIC                 # exact (Sterbenz): q = 256*n
      k = q + W                     # exact integer add (all |.| < 2^24)
      out = running_max(k)          # inclusive scan, 1 elem/cycle

  W[s*196 + i] = 16*(13-y) + (7-x) - 96 + s*2^20   (i = y*14+x, s = joint)
  packs the pixel index in k's low byte (centered so every later
  round-to-nearest is exact with no correction) and a per-joint offset
  s*2^20 > range(k) above it, so the running max at element s*196+195
  equals that joint's packed max: no per-row reduce needed, the scan's
  value at each row's last element is extracted (ACT copy, strided).

Tail (once, [128,224]): rn = round(E/256); wc = E-256*rn; fy = round(wc/16)+6;
xrp = wc-16*fy; dx = -xrp/16 + (-5.5625-tx); dy = -fy/16 + (0.8125-ty);
ACT Square+accum per partition. Host sums 8x[128,2] partials / N.

Quantization is 2^-8 (vs jnp.argmax exact): ~1% of rows hit top-2 ties and
may decode the runner-up pixel; the loss deltas are zero-mean and cancel,
measured rel err ~1e-4 (threshold 2e-2).
"""
import numpy as np

B = 16384
NJ = 14
NPIX = 196
N_CORES = 8
ROWS_PER_TILE = 1792          # 128 partitions x 14 rows
K_PER_PART = 14
N_TILES = 16                  # (B/N_CORES)*NJ / ROWS_PER_TILE

SCALE = float(2 ** 16)
MAGIC = 1.5 * 2 ** 32         # ulp = 256 -> q on the 256-grid
M3 = 1.5 * 2 ** 23            # ulp = 1   -> round-to-integer magic
PSTEP = float(2 ** 20)        # per-joint offset; > range(k) ~ 2*6.3*2^16

_STATE = {}


def _register_qpack():
    import concourse.dve_ops as dve_ops
    from concourse.dve_ops import DveOp
    from concourse.dve_spec import Spec, Src0, Src1, C0, C1, scan, lower
    from concourse.dve_spec import _has_src1 as has_src1
    from concourse.dve_uop import DveOpSpec, AluOp

    name = "QPACK_ROWMAX_ANT"
    if name in dve_ops._SUB_OPCODE_FOR_NAME:
        return next(op for op in dve_ops.OPS if op.name == name)

    def _ref(in0, in1, c0, c1, c2):
        u = (in0.astype(np.float32) * np.float32(c0)).astype(np.float32)
        u = (u + np.float32(c1)).astype(np.float32)
        q = (u - np.float32(c1)).astype(np.float32)
        k = (q + in1.astype(np.float32)).astype(np.float32)
        flat = k.reshape(k.shape[0], -1)
        return np.maximum.accumulate(flat, axis=1).reshape(in0.shape)

    u = (Src0 * C0) + C1
    q = u - C1
    k = q + Src1
    spec = Spec(body=scan(AluOp.MAX, k), reference=_ref)

    row = dve_ops._CUSTOM_DVE_ROW_BASE + len(dve_ops.OPS)
    assert row < 0x20, "custom-DVE opcode rows exhausted"
    shas = {}
    for ver in ("v3", "v4"):
        s = DveOpSpec(name=name, opcode=row, uops=lower(spec, ver=ver),
                      rd1_en=has_src1(spec))
        shas[ver] = s.sha(ver)
    op = DveOp(name, spec, subdim=False, uops_sha=shas)
    dve_ops.OPS.append(op)
    dve_ops._SUB_OPCODE_FOR_NAME[name] = row
    dve_ops.CUSTOM_DVE_SPECS[name] = spec
    return op


def _build():
    import concourse.bacc as bacc
    import concourse.mybir as mybir
    from concourse.tile import TileContext

    qpack = _register_qpack()

    F32 = mybir.dt.float32
    A = mybir.AluOpType
    AF = mybir.ActivationFunctionType

    rows = N_TILES * ROWS_PER_TILE
    ncols = N_TILES * K_PER_PART   # 224

    nc = bacc.Bacc()
    h = nc.declare_dram_parameter("h", [rows, NPIX], F32, isOutput=False)
    w = nc.declare_dram_parameter("w", [128, NJ * NPIX], F32, isOutput=False)
    tx = nc.declare_dram_parameter("tx", [128, ncols], F32, isOutput=False)
    ty = nc.declare_dram_parameter("ty", [128, ncols], F32, isOutput=False)
    out = nc.declare_dram_parameter("part", [128, 2], F32, isOutput=True)

    with TileContext(nc) as tc:
        with tc.tile_pool(name="hpool", bufs=6) as hpool, \
             tc.tile_pool(name="consts", bufs=1) as cpool, \
             tc.tile_pool(name="acc", bufs=1) as accpool:
            wt = cpool.tile([128, NJ * NPIX], F32, tag="wt")
            nc.sync.dma_start(wt[:], w[:])
            txt = cpool.tile([128, ncols], F32, tag="txt")
            nc.sync.dma_start(txt[:], tx[:])
            tyt = cpool.tile([128, ncols], F32, tag="tyt")
            nc.sync.dma_start(tyt[:], ty[:])
            kmax = accpool.tile([128, ncols], F32, tag="kmax")
            part_sb = accpool.tile([128, 2], F32, tag="part")

            w3 = wt[:].rearrange("p (s n) -> p s n", n=NPIX)
            for t in range(N_TILES):
                ht = hpool.tile([128, K_PER_PART * NPIX], F32, tag="ht")
                nc.sync.dma_start(
                    ht[:],
                    h[t * ROWS_PER_TILE:(t + 1) * ROWS_PER_TILE, :]
                    .rearrange("(p k) f -> p (k f)", p=128))
                ht3 = ht[:].rearrange("p (s n) -> p s n", n=NPIX)
                nc.vector._custom_dve(
                    qpack, out=ht3, in0=ht3, in1=w3, s0=SCALE, s1=MAGIC)
                # running max at each row's last element = that row's packed max
                nc.scalar.activation(
                    kmax[:, t * K_PER_PART:(t + 1) * K_PER_PART],
                    ht3[:, :, NPIX - 1:NPIX].rearrange("p s o -> p (s o)"),
                    AF.Identity)

            # tail decode on [128, 224]; all rounds exact by construction
            t1 = accpool.tile([128, ncols], F32, tag="t1")
            nc.vector.tensor_scalar(t1[:], kmax[:], 2.0 ** -8, M3,
                                    op0=A.mult, op1=A.add)
            rn = accpool.tile([128, ncols], F32, tag="rn")
            nc.vector.tensor_scalar(rn[:], t1[:], M3, None, op0=A.subtract)
            wc = accpool.tile([128, ncols], F32, tag="wc")
            nc.vector.scalar_tensor_tensor(wc[:], rn[:], -256.0, kmax[:],
                                           op0=A.mult, op1=A.add)
            t2 = accpool.tile([128, ncols], F32, tag="t2")
            nc.vector.tensor_scalar(t2[:], wc[:], 2.0 ** -4, M3 + 6.0,
                                    op0=A.mult, op1=A.add)
            fy = accpool.tile([128, ncols], F32, tag="fy")
            nc.vector.tensor_scalar(fy[:], t2[:], M3, None, op0=A.subtract)
            xrp = accpool.tile([128, ncols], F32, tag="xrp")
            nc.vector.scalar_tensor_tensor(xrp[:], fy[:], -16.0, wc[:],
                                           op0=A.mult, op1=A.add)
            dx = accpool.tile([128, ncols], F32, tag="dx")
            nc.vector.scalar_tensor_tensor(dx[:], xrp[:], -0.0625, txt[:],
                                           op0=A.mult, op1=A.add)
            dy = accpool.tile([128, ncols], F32, tag="dy")
            nc.vector.scalar_tensor_tensor(dy[:], fy[:], -0.0625, tyt[:],
                                           op0=A.mult, op1=A.add)
            sq = accpool.tile([128, ncols], F32, tag="sq")
            nc.scalar.activation(sq[:], dx[:], AF.Square,
                                 accum_out=part_sb[:, 0:1])
            sq2 = accpool.tile([128, ncols], F32, tag="sq2")
            nc.scalar.activation(sq2[:], dy[:], AF.Square,
                                 accum_out=part_sb[:, 1:2])

            nc.sync.dma_start(out[:], part_sb[:])
    nc.finalize()
    return nc


def _weight_pattern() -> np.ndarray:
    """W[s*196 + i] = 16*(13-y) + (7-x) - 96 + s*2^20 for i = y*14+x."""
    i = np.arange(NPIX)
    y = i // 14
    x = i % 14
    w9 = 16.0 * (13 - y) + (7.0 - x) - 96.0
    s = np.arange(NJ, dtype=np.float64)[:, None]
    wp = (w9[None, :] + s * PSTEP).astype(np.float32).reshape(-1)
    return np.broadcast_to(wp, (128, NJ * NPIX)).copy()


def _targets(t_shard: np.ndarray):
    """Per-column targets matching kmax layout (col = tile*14 + joint), with
    the decode constants folded in: txc = -5.5625 - tpx, tyc = 0.8125 - tpy."""
    b = t_shard.shape[0]
    t2 = t_shard.reshape(b, 28)
    tpx = np.ascontiguousarray(t2[:, :14]).reshape(-1)
    tpy = np.ascontiguousarray(t2[:, 14:]).reshape(-1)
    tx = tpx.reshape(N_TILES, 128, K_PER_PART).transpose(1, 0, 2).reshape(128, -1)
    ty = tpy.reshape(N_TILES, 128, K_PER_PART).transpose(1, 0, 2).reshape(128, -1)
    txc = (-5.5625 - tx).astype(np.float32)
    tyc = (0.8125 - ty).astype(np.float32)
    return np.ascontiguousarray(txc), np.ascontiguousarray(tyc)


def kernel(o: np.ndarray, h: np.ndarray, t: np.ndarray, v: np.ndarray,
           _trace: bool = False, _tmpdir: str | None = None) -> np.ndarray:
    from concourse.bass_utils import run_bass_kernel_spmd

    if "nc" not in _STATE:
        _STATE["nc"] = _build()
    nc = _STATE["nc"]

    h = np.ascontiguousarray(np.asarray(h, dtype=np.float32))
    t = np.ascontiguousarray(np.asarray(t, dtype=np.float32))
    bs = B // N_CORES
    wp = _weight_pattern()
    in_maps = []
    for c in range(N_CORES):
        h_shard = h[c * bs:(c + 1) * bs].reshape(bs * NJ, NPIX)
        txv, tyv = _targets(t[c * bs:(c + 1) * bs])
        in_maps.append({"h": h_shard, "w": wp, "tx": txv, "ty": tyv})

    res = run_bass_kernel_spmd(
        nc, in_maps, list(range(N_CORES)),
        trace=_trace, tmpdir=_tmpdir)
    _STATE["last_result"] = res
    total = np.float64(0.0)
    for c in range(N_CORES):
        total += np.asarray(res.results[c]["part"], dtype=np.float64).sum()
    n = np.float32(B * NJ)
    return np.float32(np.float32(total) / n)


# revision 9
# speedup vs baseline: 1.8720x; 1.2392x over previous
"""Trainium2 kernel for nn_MeanSquaredError2: MSE between argmax-decoded
heatmap coordinates and targets.

loss = sum_{b,j} [(px - tpx)^2 + (py - tpy)^2] / (B*NJ)
  where idx = argmax(h[b,j]), px = (idx%14)/16, py = (idx//14)/16 and
  (tpx, tpy) follow the reference's concat-then-reshape pairing of t.
Inputs o and v do not affect the result (USE_VISIBILITY=False).

Pure data parallel over 8 cores (2048 batches each). Per core, h streams in
16 tiles of [128 part x (14 rows x 196 pix)]; the whole
quantize+pack+row-argmax runs as ONE single-stream custom DVE instruction
per tile (~1.07 cyc/elem), leaving the kernel DMA-bound:

  custom op QPACK_ROWMAX_ANT (registered into dve_ops.OPS at import):
      u  = (h*2^16 + MAGIC)        # rounds h*2^16 to the 256-grid
      q  = u - MAGIC               # exact (Sterbenz): q = 256*n
      k  = (q - Idx) + PageIdx(0, 196+2^20)
                                   # == q - j + s*2^20 for in-page pixel j,
                                   #    row (joint) s; all exact (< 2^24)
      out = running_max(k)         # inclusive scan, no page reset needed:
                                   # s*2^20 > range(q-j) isolates rows

  The position rides in k's low byte (-j mod 256) and the row offset above
  the value bits, so the running max at element s*196+195 equals row s's
  packed max; that element is extracted per row (ACT copy, strided). The
  outer scan's expr contains the Idx/PageIdx scans - Scan.__post_init__
  forbids that composition, but lower() schedules it correctly (each scan
  gets its own stage with same-stage feedback); constructed unchecked and
  validated against numpy on hardware.

Tail (once, [128,224]): rn = round(E/256); wc = E-256*rn (= -j or 256-j);
j = 256*(wc>0) - wc; cand = round(j/14); xr = j-14*cand; y = cand-(xr<0);
x = xr+14*(xr<0); dx = x/16 - tx; dy = y/16 - ty; ACT Square+accum per
partition. Host sums 8x[128,2] partials / N.

Quantization is 2^-8 (vs jnp.argmax exact): ~1% of rows hit top-2 ties and
may decode the runner-up pixel; the loss deltas are zero-mean and mostly
cancel, measured rel err ~1e-3 (threshold 2e-2).
"""
import numpy as np

B = 16384
NJ = 14
NPIX = 196
N_CORES = 8
ROWS_PER_TILE = 1792          # 128 partitions x 14 rows
K_PER_PART = 14
N_TILES = 16                  # (B/N_CORES)*NJ / ROWS_PER_TILE

SCALE = float(2 ** 16)
MAGIC = 1.5 * 2 ** 32         # ulp = 256 -> q on the 256-grid
M3 = 1.5 * 2 ** 23            # ulp = 1   -> round-to-integer magic
PSTEP = float(2 ** 20)        # per-row offset; > range(q - j) ~ 2*6.3*2^16

_STATE = {}


def _register_qpack():
    import concourse.dve_ops as dve_ops
    from concourse.dve_ops import DveOp
    from concourse.dve_spec import (
        Spec, Src0, C0, C1, C2, Zero, One, Scan, lower,
    )
    from concourse.dve_spec import _has_src1 as has_src1
    from concourse.dve_uop import DveOpSpec, AluOp, AluInp

    name = "QPACK_ROWMAX_ANT"
    if name in dve_ops._SUB_OPCODE_FOR_NAME:
        return next(op for op in dve_ops.OPS if op.name == name)

    def _ref(in0, in1, c0, c1, c2):
        p = in0.shape[0]
        flat = in0.reshape(p, -1)
        n = flat.shape[1]
        u = (flat.astype(np.float32) * np.float32(c0)).astype(np.float32)
        u = (u + np.float32(c1)).astype(np.float32)
        q = (u - np.float32(c1)).astype(np.float32)
        j = (np.arange(n) % NPIX).astype(np.float32)
        s = (np.arange(n) // NPIX).astype(np.float32) * np.float32(c2 - NPIX + 1)
        k = (q + (s - j).astype(np.float32)).astype(np.float32)
        return np.maximum.accumulate(k, axis=1).reshape(in0.shape)

    u = (Src0 * C0) + C1
    q = u - C1
    # page-counter scan (PageIdx(One, C2)): holds within a page, +C2 at each
    # boundary. Its steady-state stage is patched below from BYPASS(CURR) to
    # SUBTRACT(CURR, One) so it counts -1 per element; with C2 = 195+PSTEP the
    # boundary step lands the value at (page s, elem j) on -j + s*PSTEP.
    ideg = Scan(AluOp.ADD, Zero, init=One, _subdim_step=C2)
    t2 = q + ideg
    # outer max-scan over an expr that contains the ideg scan:
    # Scan.__post_init__ rejects the composition, so construct unchecked.
    m = object.__new__(Scan)
    object.__setattr__(m, "op", AluOp.MAX)
    object.__setattr__(m, "expr", t2)
    object.__setattr__(m, "init", None)
    object.__setattr__(m, "_subdim_step", None)
    spec = Spec(body=m, reference=_ref)

    row = dve_ops._CUSTOM_DVE_ROW_BASE + len(dve_ops.OPS)
    assert row < 0x20, "custom-DVE opcode rows exhausted"
    shas = {}
    compiled = {}
    for ver in ("v3", "v4"):
        uops = lower(spec, ver=ver)
        assert len(uops) == 3, f"expected [seed, steady, step], got {len(uops)}"
        seed, steady, _step = uops
        ks = [i for i, d in enumerate(steady.datapath_config)
              if d.op == AluOp.BYPASS and d.alu_src0 == AluInp.CURR_ALU_OUT]
        assert len(ks) == 1, f"page-counter stage not unique: {ks}"
        k = ks[0]
        one_lane = seed.datapath_config[k].alu_src0  # delay lane carrying One
        assert one_lane.name.startswith("PREV_DELAY"), one_lane
        steady.datapath_config[k].op = AluOp.SUBTRACT
        steady.datapath_config[k].alu_src0 = AluInp.CURR_ALU_OUT
        steady.datapath_config[k].alu_src1 = one_lane
        s = DveOpSpec(name=name, opcode=row, uops=uops, rd1_en=has_src1(spec))
        shas[ver] = s.sha(ver)
        compiled[ver] = s
    op = DveOp(name, spec, subdim=True, uops_sha=shas)
    dve_ops.OPS.append(op)
    dve_ops._SUB_OPCODE_FOR_NAME[name] = row
    dve_ops.CUSTOM_DVE_SPECS[name] = spec
    # compile() must return the patched program, not a re-lower of the spec:
    # seed its memo cache directly.
    for ver, s in compiled.items():
        dve_ops._COMPILE_CACHE[(name, ver)] = s
    return op


def _build():
    import concourse.bacc as bacc
    import concourse.mybir as mybir
    from concourse.tile import TileContext

    qpack = _register_qpack()

    F32 = mybir.dt.float32
    A = mybir.AluOpType
    AF = mybir.ActivationFunctionType

    rows = N_TILES * ROWS_PER_TILE
    ncols = N_TILES * K_PER_PART   # 224

    nc = bacc.Bacc()
    h = nc.declare_dram_parameter("h", [rows, NPIX], F32, isOutput=False)
    tx = nc.declare_dram_parameter("tx", [128, ncols], F32, isOutput=False)
    ty = nc.declare_dram_parameter("ty", [128, ncols], F32, isOutput=False)
    out = nc.declare_dram_parameter("part", [128, 2], F32, isOutput=True)

    with TileContext(nc) as tc:
        with tc.tile_pool(name="hpool", bufs=6) as hpool, \
             tc.tile_pool(name="consts", bufs=1) as cpool, \
             tc.tile_pool(name="acc", bufs=1) as accpool:
            kmax = accpool.tile([128, ncols], F32, tag="kmax")
            part_sb = accpool.tile([128, 2], F32, tag="part")

            for t in range(N_TILES):
                ht = hpool.tile([128, K_PER_PART * NPIX], F32, tag="ht")
                nc.sync.dma_start(
                    ht[:],
                    h[t * ROWS_PER_TILE:(t + 1) * ROWS_PER_TILE, :]
                    .rearrange("(p k) f -> p (k f)", p=128))
                ht3 = ht[:].rearrange("p (s n) -> p s n", n=NPIX)
                nc.vector._custom_dve(
                    qpack, out=ht3, in0=ht3,
                    s0=SCALE, s1=MAGIC, imm2=float(NPIX) - 1.0 + PSTEP)
                # running max at each row's last element = that row's packed max
                nc.scalar.activation(
                    kmax[:, t * K_PER_PART:(t + 1) * K_PER_PART],
                    ht3[:, :, NPIX - 1:NPIX].rearrange("p s o -> p (s o)"),
                    AF.Identity)

            # targets arrive late; they are only needed by the tail
            txt = cpool.tile([128, ncols], F32, tag="txt")
            nc.sync.dma_start(txt[:], tx[:])
            tyt = cpool.tile([128, ncols], F32, tag="tyt")
            nc.sync.dma_start(tyt[:], ty[:])

            # tail decode on [128, 224]
            t1 = accpool.tile([128, ncols], F32, tag="t1")
            nc.vector.tensor_scalar(t1[:], kmax[:], 2.0 ** -8, M3,
                                    op0=A.mult, op1=A.add)
            rn = accpool.tile([128, ncols], F32, tag="rn")
            nc.vector.tensor_scalar(rn[:], t1[:], M3, None, op0=A.subtract)
            wc = accpool.tile([128, ncols], F32, tag="wc")
            nc.vector.scalar_tensor_tensor(wc[:], rn[:], -256.0, kmax[:],
                                           op0=A.mult, op1=A.add)
            m1 = accpool.tile([128, ncols], F32, tag="m1")
            nc.vector.tensor_scalar(m1[:], wc[:], 0.0, None, op0=A.is_gt)
            jj = accpool.tile([128, ncols], F32, tag="jj")
            nc.vector.scalar_tensor_tensor(jj[:], m1[:], 256.0, wc[:],
                                           op0=A.mult, op1=A.subtract)
            c14 = accpool.tile([128, ncols], F32, tag="c14")
            nc.vector.tensor_scalar(c14[:], jj[:], 1.0 / 14.0, M3,
                                    op0=A.mult, op1=A.add)
            cand = accpool.tile([128, ncols], F32, tag="cand")
            nc.vector.tensor_scalar(cand[:], c14[:], M3, None, op0=A.subtract)
            xr = accpool.tile([128, ncols], F32, tag="xr")
            nc.vector.scalar_tensor_tensor(xr[:], cand[:], -14.0, jj[:],
                                           op0=A.mult, op1=A.add)
            m2 = accpool.tile([128, ncols], F32, tag="m2")
            nc.vector.tensor_scalar(m2[:], xr[:], 0.0, None, op0=A.is_lt)
            yy = accpool.tile([128, ncols], F32, tag="yy")
            nc.vector.scalar_tensor_tensor(yy[:], m2[:], -1.0, cand[:],
                                           op0=A.mult, op1=A.add)
            xx = accpool.tile([128, ncols], F32, tag="xx")
            nc.vector.scalar_tensor_tensor(xx[:], m2[:], 14.0, xr[:],
                                           op0=A.mult, op1=A.add)
            dx = accpool.tile([128, ncols], F32, tag="dx")
            nc.vector.scalar_tensor_tensor(dx[:], xx[:], 0.0625, txt[:],
                                           op0=A.mult, op1=A.subtract)
            dy = accpool.tile([128, ncols], F32, tag="dy")
            nc.vector.scalar_tensor_tensor(dy[:], yy[:], 0.0625, tyt[:],
                                           op0=A.mult, op1=A.subtract)
            sq = accpool.tile([128, ncols], F32, tag="sq")
            nc.scalar.activation(sq[:], dx[:], AF.Square,
                                 accum_out=part_sb[:, 0:1])
            sq2 = accpool.tile([128, ncols], F32, tag="sq2")
            nc.scalar.activation(sq2[:], dy[:], AF.Square,
                                 accum_out=part_sb[:, 1:2])

            nc.sync.dma_start(out[:], part_sb[:])
    nc.finalize()
    return nc


def _targets(t_shard: np.ndarray):
    """Per-column raw targets matching kmax layout (col = tile*14 + joint)."""
    b = t_shard.shape[0]
    t2 = t_shard.reshape(b, 28)
    tpx = np.ascontiguousarray(t2[:, :14]).reshape(-1)
    tpy = np.ascontiguousarray(t2[:, 14:]).reshape(-1)
    tx = tpx.reshape(N_TILES, 128, K_PER_PART).transpose(1, 0, 2).reshape(128, -1)
    ty = tpy.reshape(N_TILES, 128, K_PER_PART).transpose(1, 0, 2).reshape(128, -1)
    return np.ascontiguousarray(tx), np.ascontiguousarray(ty)


def kernel(o: np.ndarray, h: np.ndarray, t: np.ndarray, v: np.ndarray,
           _trace: bool = False, _tmpdir: str | None = None) -> np.ndarray:
    from concourse.bass_utils import run_bass_kernel_spmd

    if "nc" not in _STATE:
        _STATE["nc"] = _build()
    nc = _STATE["nc"]

    h = np.ascontiguousarray(np.asarray(h, dtype=np.float32))
    t = np.ascontiguousarray(np.asarray(t, dtype=np.float32))
    bs = B // N_CORES
    in_maps = []
    for c in range(N_CORES):
        h_shard = h[c * bs:(c + 1) * bs].reshape(bs * NJ, NPIX)
        txv, tyv = _targets(t[c * bs:(c + 1) * bs])
        in_maps.append({"h": h_shard, "tx": txv, "ty": tyv})

    res = run_bass_kernel_spmd(
        nc, in_maps, list(range(N_CORES)),
        trace=_trace, tmpdir=_tmpdir)
    _STATE["last_result"] = res
    total = np.float64(0.0)
    for c in range(N_CORES):
        total += np.asarray(res.results[c]["part"], dtype=np.float64).sum()
    n = np.float32(B * NJ)
    return np.float32(np.float32(total) / n)
